# revision 21
# baseline (speedup 1.0000x reference)
"""LoRA grouped-experts MoE MLP on 8 NeuronCores (expert-parallel).

Each core computes one expert's full MLP:
    g = silu(x @ Wg + (x @ Ag) @ (s*Bg))
    u =       x @ Wu + (x @ Au) @ (s*Bu)
    h = g * u
    o =       h @ Wd + (h @ Ad) @ (s*Bd)

Device layout (per core):
  - x is pre-transposed on host to xT [D, T] so the contraction dim D lands
    on SBUF partitions for both matmul operands (fp32 has no DMA transpose).
  - Layer 1 computes hT [H, T] (H on partitions). Layer 2 keeps the weight
    slices stationary and produces outT [D, T]; the host transposes back.
  - All matmul inputs are bf16 (cast on host); PSUM accumulates fp32.
  - LoRA rank padded 16->32; lora B pre-scaled by alpha/rank. The LoRA
    contribution is accumulated into the same PSUM group as the base matmul.
  - Every stationary (lhsT) operand feeds two back-to-back matmuls into two
    PSUM banks (the two 512-token halves): HW-measured 112 ns/MM paired vs
    231 ns unpaired (N=512 bf16) -- the weight load otherwise serializes
    with the matmul stream.
  - Both layers stream weights through one shared slab pool so layer-2
    prefetch begins while layer-1 drains.
"""

import os

import numpy as np
import ml_dtypes

import concourse.bacc as bacc
import concourse.mybir as mybir
import concourse.tile as tile
from concourse.bass import ts
from concourse.bass_utils import run_bass_kernel_spmd

P = 128
E, D, H, R, T = 8, 2048, 4096, 16, 1024
RP = 32  # padded lora rank (K>=32 for PE matmuls)
DO = D // P   # 16
HO = H // P   # 32
ALPHA = 32.0
BF16 = mybir.dt.bfloat16
F32 = mybir.dt.float32

_NC_CACHE = []
LAST_RESULT = None

NSPLIT = int(os.environ.get("KERNEL_NSPLIT", "4"))
WBUFS = int(os.environ.get("KERNEL_WBUFS", "4"))


def _build_nc(reps=1):
    nc = bacc.Bacc("TRN2", target_bir_lowering=False, debug=False, num_devices=E)

    xT = nc.dram_tensor("xT", (D, T), BF16, kind="ExternalInput").ap()
    wg = nc.dram_tensor("wg", (D, H), BF16, kind="ExternalInput").ap()
    wu = nc.dram_tensor("wu", (D, H), BF16, kind="ExternalInput").ap()
    wd = nc.dram_tensor("wd", (H, D), BF16, kind="ExternalInput").ap()
    ag = nc.dram_tensor("ag", (D, RP), BF16, kind="ExternalInput").ap()
    bg = nc.dram_tensor("bg", (RP, H), BF16, kind="ExternalInput").ap()
    au = nc.dram_tensor("au", (D, RP), BF16, kind="ExternalInput").ap()
    bu = nc.dram_tensor("bu", (RP, H), BF16, kind="ExternalInput").ap()
    ad = nc.dram_tensor("ad", (H, RP), BF16, kind="ExternalInput").ap()
    bd = nc.dram_tensor("bd", (RP, D), BF16, kind="ExternalInput").ap()
    out = nc.dram_tensor("out", (D, T), F32, kind="ExternalOutput").ap()

    aps = dict(
        xT_r=xT.rearrange("(o p) t -> p o t", p=P),
        wg_r=wg.rearrange("(o p) h -> p o h", p=P),
        wu_r=wu.rearrange("(o p) h -> p o h", p=P),
        wd_r=wd.rearrange("(o p) d -> p o d", p=P),
        ag_r=ag.rearrange("(o p) r -> p o r", p=P),
        au_r=au.rearrange("(o p) r -> p o r", p=P),
        ad_r=ad.rearrange("(o p) r -> p o r", p=P),
        out_r=out.rearrange("(o p) t -> p o t", p=P),
        bg=bg, bu=bu, bd=bd,
    )

    with tile.TileContext(nc) as tc:
        with (
            tc.tile_pool(name="persist", bufs=1) as pp,
            tc.tile_pool(name="stage", bufs=3) as sp,
            tc.tile_pool(name="wpool", bufs=WBUFS) as wp,
            tc.tile_pool(name="lslab", bufs=2) as lp,
            tc.tile_pool(name="psum", bufs=8, space="PSUM") as psp,
        ):
            for rep in range(reps):
                _emit(nc, tc, pp, sp, wp, lp, psp, aps, rep)

    nc.compile()
    return nc


def _dma_split(nc, dst, src, n):
    """Split a [P, O, F] slab load into n dma_starts over the O axis."""
    n = max(1, min(n, NSPLIT)) if NSPLIT > 0 else 1
    o = dst.shape[1]
    step = o // n
    for i in range(n):
        nc.sync.dma_start(dst[:, ts(i, step), :], src[:, ts(i, step), :])


def _emit(nc, tc, pp, sp, wp, lp, psp, aps, rep):
    xT_r, wg_r, wu_r, wd_r = aps["xT_r"], aps["wg_r"], aps["wu_r"], aps["wd_r"]
    ag_r, au_r, ad_r = aps["ag_r"], aps["au_r"], aps["ad_r"]
    bg, bu, bd, out_r = aps["bg"], aps["bu"], aps["bd"], aps["out_r"]

    hT_sb = pp.tile([P, HO, T], BF16, tag="hT")
    ag_sb = pp.tile([P, DO, RP], BF16, tag="ag")
    au_sb = pp.tile([P, DO, RP], BF16, tag="au")
    ad_sb = pp.tile([P, HO, RP], BF16, tag="ad")
    bd_sb = pp.tile([RP, D], BF16, tag="bd")
    aTg_sb = pp.tile([RP, T], BF16, tag="aTg")
    aTu_sb = pp.tile([RP, T], BF16, tag="aTu")
    aTd_sb = pp.tile([RP, T], BF16, tag="aTd")

    nc.sync.dma_start(ag_sb[:], ag_r[:])
    nc.sync.dma_start(au_sb[:], au_r[:])
    nc.sync.dma_start(ad_sb[:], ad_r[:])
    nc.sync.dma_start(bd_sb[:], bd[:])

    with tc.tile_pool(name=f"xpool{rep}", bufs=1) as xp:
        xT_sb = xp.tile([P, DO, T], BF16, tag="xT")
        _dma_split(nc, xT_sb, xT_r, 4)

        # aT = (x @ A)^T for gate/up (scale folded into B on host)
        for a_sb, aT_sb in ((ag_sb, aTg_sb), (au_sb, aTu_sb)):
            pa0 = psp.tile([RP, 512], F32, tag="mm")
            pa1 = psp.tile([RP, 512], F32, tag="mm")
            for o in range(DO):
                st, sp_ = (o == 0), (o == DO - 1)
                nc.tensor.matmul(pa0[:], a_sb[:, o, :], xT_sb[:, o, 0:512],
                                 start=st, stop=sp_)
                nc.tensor.matmul(pa1[:], a_sb[:, o, :], xT_sb[:, o, 512:1024],
                                 start=st, stop=sp_)
            nc.vector.tensor_copy(aT_sb[:, 0:512], pa0[:])
            nc.vector.tensor_copy(aT_sb[:, 512:1024], pa1[:])

        # layer 1: hT[h, t] = silu(gate) * up; lhsT paired over t-halves
        for j in range(H // 512):
            wg_t = wp.tile([P, DO, 512], BF16, tag="w")
            _dma_split(nc, wg_t, wg_r[:, :, ts(j, 512)], 4)
            wu_t = wp.tile([P, DO, 512], BF16, tag="w")
            _dma_split(nc, wu_t, wu_r[:, :, ts(j, 512)], 4)
            bg_t = lp.tile([RP, 512], BF16, tag="bgj")
            nc.sync.dma_start(bg_t[:], bg[:, ts(j, 512)])
            bu_t = lp.tile([RP, 512], BF16, tag="buj")
            nc.sync.dma_start(bu_t[:], bu[:, ts(j, 512)])
            for hsub in range(4):
                hc = j * 4 + hsub

                def l1_proj(w_t, b_t, aT_sb):
                    p0 = psp.tile([P, 512], F32, tag="mm")
                    p1 = psp.tile([P, 512], F32, tag="mm")
                    for o in range(DO):
                        st = (o == 0)
                        nc.tensor.matmul(p0[:], w_t[:, o, ts(hsub, P)],
                                         xT_sb[:, o, 0:512],
                                         start=st, stop=False)
                        nc.tensor.matmul(p1[:], w_t[:, o, ts(hsub, P)],
                                         xT_sb[:, o, 512:1024],
                                         start=st, stop=False)
                    nc.tensor.matmul(p0[:], b_t[:, ts(hsub, P)],
                                     aT_sb[:, 0:512], start=False, stop=True)
                    nc.tensor.matmul(p1[:], b_t[:, ts(hsub, P)],
                                     aT_sb[:, 512:1024], start=False, stop=True)
                    return p0, p1

                pg0, pg1 = l1_proj(wg_t, bg_t, aTg_sb)
                pu0, pu1 = l1_proj(wu_t, bu_t, aTu_sb)
                for t, pg_, pu_ in ((0, pg0, pu0), (1, pg1, pu1)):
                    g_act = sp.tile([P, 512], F32, tag="gact")
                    nc.scalar.activation(
                        g_act[:], pg_[:], mybir.ActivationFunctionType.Silu)
                    nc.vector.tensor_mul(
                        hT_sb[:, hc, ts(t, 512)], g_act[:], pu_[:])

    # aTd = (h @ Ad)^T, lhsT paired over t-halves
    pa0 = psp.tile([RP, 512], F32, tag="mm")
    pa1 = psp.tile([RP, 512], F32, tag="mm")
    for hc in range(HO):
        st, sp_ = (hc == 0), (hc == HO - 1)
        nc.tensor.matmul(pa0[:], ad_sb[:, hc, :], hT_sb[:, hc, 0:512],
                         start=st, stop=sp_)
        nc.tensor.matmul(pa1[:], ad_sb[:, hc, :], hT_sb[:, hc, 512:1024],
                         start=st, stop=sp_)
    nc.vector.tensor_copy(aTd_sb[:, 0:512], pa0[:])
    nc.vector.tensor_copy(aTd_sb[:, 512:1024], pa1[:])

    # layer 2: outT[d, t] = (h @ Wd + lora)^T; weight slices stationary,
    # paired over t-halves.
    for k in range(D // 512):
        s0 = wp.tile([P, DO, 512], BF16, tag="w")
        _dma_split(nc, s0, wd_r[:, 0:16, ts(k, 512)], 4)
        s1 = wp.tile([P, DO, 512], BF16, tag="w")
        _dma_split(nc, s1, wd_r[:, 16:32, ts(k, 512)], 4)
        for dsub in range(4):
            dd = k * 4 + dsub  # global 128-wide d-chunk
            po0 = psp.tile([P, 512], F32, tag="mm")
            po1 = psp.tile([P, 512], F32, tag="mm")
            for hc in range(HO):
                st = (hc == 0)
                lhsT = (s0 if hc < 16 else s1)[:, hc % 16, ts(dsub, P)]
                nc.tensor.matmul(po0[:], lhsT, hT_sb[:, hc, 0:512],
                                 start=st, stop=False)
                nc.tensor.matmul(po1[:], lhsT, hT_sb[:, hc, 512:1024],
                                 start=st, stop=False)
            nc.tensor.matmul(po0[:], bd_sb[:, ts(dd, P)], aTd_sb[:, 0:512],
                             start=False, stop=True)
            nc.tensor.matmul(po1[:], bd_sb[:, ts(dd, P)], aTd_sb[:, 512:1024],
                             start=False, stop=True)
            for t, po_ in ((0, po0), (1, po1)):
                o_t = sp.tile([P, 512], F32, tag="ostage")
                nc.scalar.copy(o_t[:], po_[:])
                nc.sync.dma_start(out_r[:, dd, ts(t, 512)], o_t[:])


def _build_nc2(reps=1, wsplit=2, xsplit=4, preload_b=False, wbufs=None):
    """v2: host-swizzled DRAM layouts -> every DMA descriptor is a fat
    contiguous per-partition line (8-32KB vs 1-2KB in v1).
    v3 = wsplit=1, xsplit=1, preload_b=True, wbufs=3 (descriptor-lean)."""
    nc = bacc.Bacc("TRN2", target_bir_lowering=False, debug=False, num_devices=E)

    HJ = H // 512  # 8 weight slabs for layer 1
    xTd = nc.dram_tensor("xTd", (P, DO * T), BF16, kind="ExternalInput").ap()
    wgd = nc.dram_tensor("wgd", (HJ, P, DO * 512), BF16, kind="ExternalInput").ap()
    wud = nc.dram_tensor("wud", (HJ, P, DO * 512), BF16, kind="ExternalInput").ap()
    wdd = nc.dram_tensor("wdd", (4, 2, P, 16 * 512), BF16, kind="ExternalInput").ap()
    agd = nc.dram_tensor("agd", (P, DO * RP), BF16, kind="ExternalInput").ap()
    aud = nc.dram_tensor("aud", (P, DO * RP), BF16, kind="ExternalInput").ap()
    add = nc.dram_tensor("add", (P, HO * RP), BF16, kind="ExternalInput").ap()
    bg = nc.dram_tensor("bg", (RP, H), BF16, kind="ExternalInput").ap()
    bu = nc.dram_tensor("bu", (RP, H), BF16, kind="ExternalInput").ap()
    bd = nc.dram_tensor("bd", (RP, D), BF16, kind="ExternalInput").ap()
    outd = nc.dram_tensor("outd", (4, 2, P, 2 * T), F32, kind="ExternalOutput").ap()

    aps = dict(
        xT_r=xTd.rearrange("p (o t) -> p o t", o=DO),
        wg_r=wgd.rearrange("j p (o c) -> j p o c", o=DO),
        wu_r=wud.rearrange("j p (o c) -> j p o c", o=DO),
        wd_r=wdd.rearrange("k h p (o c) -> k h p o c", o=16),
        ag_r=agd.rearrange("p (o r) -> p o r", o=DO),
        au_r=aud.rearrange("p (o r) -> p o r", o=DO),
        ad_r=add.rearrange("p (o r) -> p o r", o=HO),
        out_r=outd.rearrange("k h p (d t) -> k h p d t", d=2),
        bg=bg, bu=bu, bd=bd,
    )

    with tile.TileContext(nc) as tc:
        with (
            tc.tile_pool(name="persist", bufs=1) as pp,
            tc.tile_pool(name="stage", bufs=3) as sp,
            tc.tile_pool(name="wpool", bufs=wbufs or WBUFS) as wp,
            tc.tile_pool(name="lslab", bufs=2) as lp,
            tc.tile_pool(name="ostage", bufs=2) as op,
            tc.tile_pool(name="psum", bufs=8, space="PSUM") as psp,
        ):
            for rep in range(reps):
                _emit2(nc, tc, pp, sp, wp, lp, op, psp, aps, rep, wsplit,
                       xsplit, preload_b)

    nc.compile()
    return nc


def _dma_osplit(nc, dst, src, n):
    """Split a [P, O, F] slab load into n dma_starts over the O axis."""
    o = dst.shape[1]
    n = max(1, min(n, o))
    step = o // n
    for i in range(n):
        nc.sync.dma_start(dst[:, ts(i, step), :], src[:, ts(i, step), :])


def _emit2(nc, tc, pp, sp, wp, lp, op, psp, aps, rep, wsplit,
           xsplit=4, preload_b=False):
    xT_r, wg_r, wu_r, wd_r = aps["xT_r"], aps["wg_r"], aps["wu_r"], aps["wd_r"]
    ag_r, au_r, ad_r = aps["ag_r"], aps["au_r"], aps["ad_r"]
    bg, bu, bd, out_r = aps["bg"], aps["bu"], aps["bd"], aps["out_r"]

    hT_sb = pp.tile([P, HO, T], BF16, tag="hT")
    ag_sb = pp.tile([P, DO, RP], BF16, tag="ag")
    au_sb = pp.tile([P, DO, RP], BF16, tag="au")
    ad_sb = pp.tile([P, HO, RP], BF16, tag="ad")
    bd_sb = pp.tile([RP, D], BF16, tag="bd")
    aTg_sb = pp.tile([RP, T], BF16, tag="aTg")
    aTu_sb = pp.tile([RP, T], BF16, tag="aTu")
    aTd_sb = pp.tile([RP, T], BF16, tag="aTd")

    nc.sync.dma_start(ag_sb[:], ag_r[:])
    nc.sync.dma_start(au_sb[:], au_r[:])
    nc.sync.dma_start(ad_sb[:], ad_r[:])
    nc.sync.dma_start(bd_sb[:], bd[:])
    bg_sb = bu_sb = None
    if preload_b:
        bg_sb = pp.tile([RP, H], BF16, tag="bgall")
        bu_sb = pp.tile([RP, H], BF16, tag="buall")
        nc.sync.dma_start(bg_sb[:], bg[:])
        nc.sync.dma_start(bu_sb[:], bu[:])

    with tc.tile_pool(name=f"xpool{rep}", bufs=1) as xp:
        xT_sb = xp.tile([P, DO, T], BF16, tag="xT")
        _dma_osplit(nc, xT_sb, xT_r, xsplit)

        # aT = (x @ A)^T for gate/up (scale folded into B on host)
        for a_sb, aT_sb in ((ag_sb, aTg_sb), (au_sb, aTu_sb)):
            pa0 = psp.tile([RP, 512], F32, tag="mm")
            pa1 = psp.tile([RP, 512], F32, tag="mm")
            for o in range(DO):
                st, sp_ = (o == 0), (o == DO - 1)
                nc.tensor.matmul(pa0[:], a_sb[:, o, :], xT_sb[:, o, 0:512],
                                 start=st, stop=sp_)
                nc.tensor.matmul(pa1[:], a_sb[:, o, :], xT_sb[:, o, 512:1024],
                                 start=st, stop=sp_)
            nc.vector.tensor_copy(aT_sb[:, 0:512], pa0[:])
            nc.vector.tensor_copy(aT_sb[:, 512:1024], pa1[:])

        # layer 1: hT[h, t] = silu(gate) * up; lhsT paired over t-halves
        for j in range(H // 512):
            wg_t = wp.tile([P, DO, 512], BF16, tag="w")
            _dma_osplit(nc, wg_t, wg_r[j], wsplit)
            wu_t = wp.tile([P, DO, 512], BF16, tag="w")
            _dma_osplit(nc, wu_t, wu_r[j], wsplit)
            if preload_b:
                bg_t, bu_t, boff = bg_sb, bu_sb, j * 512
            else:
                bg_t = lp.tile([RP, 512], BF16, tag="bgj")
                nc.sync.dma_start(bg_t[:], bg[:, ts(j, 512)])
                bu_t = lp.tile([RP, 512], BF16, tag="buj")
                nc.sync.dma_start(bu_t[:], bu[:, ts(j, 512)])
                boff = 0
            for hsub in range(4):
                hc = j * 4 + hsub

                def l1_proj(w_t, b_t, aT_sb):
                    bs = boff + hsub * P
                    p0 = psp.tile([P, 512], F32, tag="mm")
                    p1 = psp.tile([P, 512], F32, tag="mm")
                    for o in range(DO):
                        st = (o == 0)
                        nc.tensor.matmul(p0[:], w_t[:, o, ts(hsub, P)],
                                         xT_sb[:, o, 0:512],
                                         start=st, stop=False)
                        nc.tensor.matmul(p1[:], w_t[:, o, ts(hsub, P)],
                                         xT_sb[:, o, 512:1024],
                                         start=st, stop=False)
                    nc.tensor.matmul(p0[:], b_t[:, bs:bs + P],
                                     aT_sb[:, 0:512], start=False, stop=True)
                    nc.tensor.matmul(p1[:], b_t[:, bs:bs + P],
                                     aT_sb[:, 512:1024], start=False, stop=True)
                    return p0, p1

                pg0, pg1 = l1_proj(wg_t, bg_t, aTg_sb)
                pu0, pu1 = l1_proj(wu_t, bu_t, aTu_sb)
                for t, pg_, pu_ in ((0, pg0, pu0), (1, pg1, pu1)):
                    g_act = sp.tile([P, 512], F32, tag="gact")
                    nc.scalar.activation(
                        g_act[:], pg_[:], mybir.ActivationFunctionType.Silu)
                    nc.vector.tensor_mul(
                        hT_sb[:, hc, ts(t, 512)], g_act[:], pu_[:])

    # aTd = (h @ Ad)^T, lhsT paired over t-halves
    pa0 = psp.tile([RP, 512], F32, tag="mm")
    pa1 = psp.tile([RP, 512], F32, tag="mm")
    for hc in range(HO):
        st, sp_ = (hc == 0), (hc == HO - 1)
        nc.tensor.matmul(pa0[:], ad_sb[:, hc, :], hT_sb[:, hc, 0:512],
                         start=st, stop=sp_)
        nc.tensor.matmul(pa1[:], ad_sb[:, hc, :], hT_sb[:, hc, 512:1024],
                         start=st, stop=sp_)
    nc.vector.tensor_copy(aTd_sb[:, 0:512], pa0[:])
    nc.vector.tensor_copy(aTd_sb[:, 512:1024], pa1[:])

    # layer 2: outT[d, t] = (h @ Wd + lora)^T; weight slices stationary,
    # paired over t-halves. Output staged as [P, 2, T] then one fat store.
    for k in range(D // 512):
        s0 = wp.tile([P, 16, 512], BF16, tag="w")
        _dma_osplit(nc, s0, wd_r[k, 0], wsplit)
        s1 = wp.tile([P, 16, 512], BF16, tag="w")
        _dma_osplit(nc, s1, wd_r[k, 1], wsplit)
        o_t = None
        for dsub in range(4):
            dd = k * 4 + dsub  # global 128-wide d-chunk
            if dsub % 2 == 0:
                o_t = op.tile([P, 2, T], F32, tag="ostage")
            po0 = psp.tile([P, 512], F32, tag="mm")
            po1 = psp.tile([P, 512], F32, tag="mm")
            for hc in range(HO):
                st = (hc == 0)
                lhsT = (s0 if hc < 16 else s1)[:, hc % 16, ts(dsub, P)]
                nc.tensor.matmul(po0[:], lhsT, hT_sb[:, hc, 0:512],
                                 start=st, stop=False)
                nc.tensor.matmul(po1[:], lhsT, hT_sb[:, hc, 512:1024],
                                 start=st, stop=False)
            nc.tensor.matmul(po0[:], bd_sb[:, ts(dd, P)], aTd_sb[:, 0:512],
                             start=False, stop=True)
            nc.tensor.matmul(po1[:], bd_sb[:, ts(dd, P)], aTd_sb[:, 512:1024],
                             start=False, stop=True)
            nc.scalar.copy(o_t[:, dsub % 2, 0:512], po0[:])
            nc.scalar.copy(o_t[:, dsub % 2, 512:1024], po1[:])
            if dsub % 2 == 1:
                nc.sync.dma_start(out_r[k, dsub // 2], o_t[:])


def make_in_maps2(x, gate_proj, up_proj, down_proj, lga, lgb, lua, lub,
                  lda, ldb):
    """Host-side shard/cast/swizzle prep for the v2 fat-descriptor layout."""
    bf = ml_dtypes.bfloat16
    scale = ALPHA / R
    x = np.asarray(x, np.float32).reshape(E, T, D)

    def pad_b(b):
        o = np.zeros((RP, b.shape[1]), np.float32)
        o[:R] = scale * b
        return o.astype(bf)

    def sw_a(a):  # [in, R] -> [P, (o R)] padded to RP
        i = a.shape[0]
        o = np.zeros((i, RP), np.float32)
        o[:, :R] = a
        return np.ascontiguousarray(
            o.reshape(i // P, P, RP).transpose(1, 0, 2).reshape(P, -1)
        ).astype(bf)

    def sw_w1(w):  # [D, H] -> [HJ, P, (o c)]
        return np.ascontiguousarray(
            w.reshape(DO, P, H // 512, 512).transpose(2, 1, 0, 3)
            .reshape(H // 512, P, DO * 512)).astype(bf)

    def sw_wd(w):  # [H, D] -> [4, 2, P, (o c)]
        return np.ascontiguousarray(
            w.reshape(2, 16, P, 4, 512).transpose(3, 0, 2, 1, 4)
            .reshape(4, 2, P, 16 * 512)).astype(bf)

    in_maps = []
    for e in range(E):
        xT = np.ascontiguousarray(x[e].T)  # [D, T]
        in_maps.append({
            "xTd": np.ascontiguousarray(
                xT.reshape(DO, P, T).transpose(1, 0, 2).reshape(P, DO * T)
            ).astype(bf),
            "wgd": sw_w1(np.asarray(gate_proj[e], np.float32)),
            "wud": sw_w1(np.asarray(up_proj[e], np.float32)),
            "wdd": sw_wd(np.asarray(down_proj[e], np.float32)),
            "agd": sw_a(np.asarray(lga[e], np.float32)),
            "aud": sw_a(np.asarray(lua[e], np.float32)),
            "add": sw_a(np.asarray(lda[e], np.float32)),
            "bg": pad_b(np.asarray(lgb[e], np.float32)),
            "bu": pad_b(np.asarray(lub[e], np.float32)),
            "bd": pad_b(np.asarray(ldb[e], np.float32)),
        })
    return in_maps


def unswizzle_out2(outd):
    """outd [4, 2, P, 2*T] f32 -> out [T, D] for one expert."""
    o = outd.reshape(4, 2, P, 2, T).transpose(0, 1, 3, 2, 4).reshape(D, T)
    return np.ascontiguousarray(o.T)


def _build_tiny(big_unused=False, num_devices=E):
    """Trivial NEFF: one small DMA in, one out. Overhead probe."""
    nc = bacc.Bacc("TRN2", target_bir_lowering=False, debug=False,
                   num_devices=num_devices)
    if big_unused is True:
        HJ = H // 512
        xT = nc.dram_tensor("xTd", (P, DO * T), BF16, kind="ExternalInput").ap()
        nc.dram_tensor("wgd", (HJ, P, DO * 512), BF16, kind="ExternalInput")
        nc.dram_tensor("wud", (HJ, P, DO * 512), BF16, kind="ExternalInput")
        nc.dram_tensor("wdd", (4, 2, P, 16 * 512), BF16, kind="ExternalInput")
        out = nc.dram_tensor("outd", (4, 2, P, 2 * T), F32,
                             kind="ExternalOutput").ap()
        xr = xT.rearrange("p (o t) -> p o t", o=DO)
        our = out[0, 0]
        with tile.TileContext(nc) as tc:
            with tc.tile_pool(name="s", bufs=1) as spp:
                t0 = spp.tile([P, 128], BF16, tag="t")
                t1 = spp.tile([P, 128], F32, tag="t2")
                nc.sync.dma_start(t0[:], xr[:, 0, 0:128])
                nc.vector.tensor_copy(t1[:], t0[:])
                nc.sync.dma_start(our[:, 0:128], t1[:])
        nc.compile()
        return nc
    xT = nc.dram_tensor("xT", (D, T), BF16, kind="ExternalInput").ap()
    out = nc.dram_tensor("out", (D, T), F32, kind="ExternalOutput").ap()
    cw = None
    if big_unused == "const":
        cdata = np.zeros((24, P, 8192), ml_dtypes.bfloat16)  # 48MB const
        cw = nc.inline_tensor(cdata, name="cw").ap()
    xr = xT.rearrange("(o p) t -> p o t", p=P)
    our = out.rearrange("(o p) t -> p o t", p=P)
    with tile.TileContext(nc) as tc:
        with tc.tile_pool(name="s", bufs=1) as sp:
            t0 = sp.tile([P, 128], BF16, tag="t")
            t1 = sp.tile([P, 128], F32, tag="t2")
            nc.sync.dma_start(t0[:], xr[:, 0, 0:128])
            if cw is not None:
                tc0 = sp.tile([P, 128], BF16, tag="tc")
                tc1 = sp.tile([P, 128], F32, tag="tc2")
                nc.sync.dma_start(tc0[:], cw[0, :, 0:128])
                nc.vector.tensor_copy(tc1[:], tc0[:])
                nc.sync.dma_start(our[:, 1, 0:128], tc1[:])
            nc.vector.tensor_copy(t1[:], t0[:])
            nc.sync.dma_start(our[:, 0, 0:128], t1[:])
    nc.compile()
    return nc


def _build_nano(num_devices=E):
    """Minimal fully-read/fully-written NEFF: pure floor probe."""
    nc = bacc.Bacc("TRN2", target_bir_lowering=False, debug=False,
                   num_devices=num_devices)
    xi = nc.dram_tensor("xi", (P, 128), BF16, kind="ExternalInput").ap()
    out = nc.dram_tensor("out", (P, 128), F32, kind="ExternalOutput").ap()
    with tile.TileContext(nc) as tc:
        with tc.tile_pool(name="s", bufs=1) as spp:
            t0 = spp.tile([P, 128], BF16, tag="t")
            t1 = spp.tile([P, 128], F32, tag="t2")
            nc.sync.dma_start(t0[:], xi[:])
            nc.vector.tensor_copy(t1[:], t0[:])
            nc.sync.dma_start(out[:], t1[:])
    nc.compile()
    return nc


def build_variant(name):
    if name.startswith("nano"):
        n = int(name.split("@")[1]) if "@" in name else E
        return _build_nano(num_devices=n)
    if name == "tiny":
        return _build_tiny()
    if name == "tiny2":
        return _build_tiny(big_unused=True)
    if name == "tinyc":
        return _build_tiny(big_unused="const")
    if name.startswith("tiny@"):
        return _build_tiny(num_devices=int(name.split("@")[1]))
    if name == "v2":
        return _build_nc2()
    if name == "v3":
        return _build_nc2(wsplit=1, xsplit=1, preload_b=True, wbufs=3)
    if name.startswith("v2r"):
        return _build_nc2(reps=int(name[3:]))
    raise KeyError(name)


def _get_nc():
    if not _NC_CACHE:
        _NC_CACHE.append(
            _build_nc2(wsplit=1, xsplit=1, preload_b=True, wbufs=3))
    return _NC_CACHE[0]


def make_in_maps(x, gate_proj, up_proj, down_proj, lga, lgb, lua, lub, lda, ldb):
    """Host-side shard/cast prep, shared by kernel() and the bench harness."""
    bf = ml_dtypes.bfloat16
    scale = ALPHA / R
    x = np.asarray(x, np.float32).reshape(E, T, D)

    def pad_a(a):
        o = np.zeros((a.shape[0], RP), np.float32)
        o[:, :R] = a
        return o.astype(bf)

    def pad_b(b):
        o = np.zeros((RP, b.shape[1]), np.float32)
        o[:R] = scale * b
        return o.astype(bf)

    in_maps = []
    for e in range(E):
        in_maps.append({
            "xT": np.ascontiguousarray(x[e].T).astype(bf),
            "wg": np.asarray(gate_proj[e], np.float32).astype(bf),
            "wu": np.asarray(up_proj[e], np.float32).astype(bf),
            "wd": np.asarray(down_proj[e], np.float32).astype(bf),
            "ag": pad_a(np.asarray(lga[e], np.float32)),
            "bg": pad_b(np.asarray(lgb[e], np.float32)),
            "au": pad_a(np.asarray(lua[e], np.float32)),
            "bu": pad_b(np.asarray(lub[e], np.float32)),
            "ad": pad_a(np.asarray(lda[e], np.float32)),
            "bd": pad_b(np.asarray(ldb[e], np.float32)),
        })
    return in_maps


def kernel(x, num_tokens_per_expert, gate_proj, up_proj, down_proj,
           lora_gate_a, lora_gate_b, lora_up_a, lora_up_b,
           lora_down_a, lora_down_b):
    global LAST_RESULT
    in_maps = make_in_maps2(x, gate_proj, up_proj, down_proj,
                            lora_gate_a, lora_gate_b, lora_up_a, lora_up_b,
                            lora_down_a, lora_down_b)
    # The axon NTFF profile hook is unavailable in this container; force the
    # no-trace PJRT path regardless of ambient BASS_TRACE.
    os.environ["BASS_NEVER_TRACE"] = "1"
    nc = _get_nc()
    res = run_bass_kernel_spmd(nc, in_maps, core_ids=list(range(E)))
    LAST_RESULT = res
    # outputs are swizzled outT per expert; unswizzle back to [T, D]
    return np.concatenate(
        [unswizzle_out2(r["outd"]) for r in res.results], axis=0)



# revision 24
# speedup vs baseline: 1.0922x; 1.0922x over previous
"""LoRA grouped-experts MoE MLP on 8 NeuronCores (expert-parallel).

Each core computes one expert's full MLP:
    g = silu(x @ Wg + (x @ Ag) @ (s*Bg))
    u =       x @ Wu + (x @ Au) @ (s*Bu)
    h = g * u
    o =       h @ Wd + (h @ Ad) @ (s*Bd)

Device layout (per core), shipped kernel = _build_nc2 "v3b":
  - x is pre-transposed on host to xT [D, T] so the contraction dim D lands
    on SBUF partitions for both matmul operands (fp32 has no DMA transpose).
  - Layer 1 computes hT [H, T] (H on partitions). Layer 2 keeps the weight
    slices stationary and produces outT [D, T]; the host un-swizzles back.
  - All matmul inputs are bf16 (cast on host); PSUM accumulates fp32;
    the DRAM output is bf16 (host casts to fp32; adds ~2e-3 rel err in
    quadrature, total ~4.4e-3 vs the 2e-2 gate).
  - LoRA rank padded 16->32; lora B pre-scaled by alpha/rank. The LoRA
    contribution is accumulated into the same PSUM group as the base matmul.
  - Every stationary (lhsT) operand feeds two back-to-back matmuls into two
    PSUM banks (the two 512-token halves): HW-measured 112 ns/MM paired vs
    231 ns unpaired (N=512 bf16) -- the weight load otherwise serializes
    with the matmul stream.
  - All DRAM tensors are host-swizzled (make_in_maps2) so every DMA lands
    as fat contiguous per-partition lines (8-32KB descriptors, ~4.7k per
    call, vs 1-2KB and ~50k in the original layout). In this axon/fakenrt
    bench environment the measured per-call time is dominated by
    per-descriptor overhead (~0.1us each), not device FLOPs: the v1 layout
    measured ~10ms/call, this layout ~5.3ms in uncontended windows
    (TimelineSim predicts 750us of actual device time for both).
  - Measurements are window-sensitive: under service contention every
    variant (even a no-op NEFF) flattens to ~10-12ms/call.
"""

import os

import numpy as np
import ml_dtypes

import concourse.bacc as bacc
import concourse.mybir as mybir
import concourse.tile as tile
from concourse.bass import ts
from concourse.bass_utils import run_bass_kernel_spmd

P = 128
E, D, H, R, T = 8, 2048, 4096, 16, 1024
RP = 32  # padded lora rank (K>=32 for PE matmuls)
DO = D // P   # 16
HO = H // P   # 32
ALPHA = 32.0
BF16 = mybir.dt.bfloat16
F32 = mybir.dt.float32

_NC_CACHE = []
LAST_RESULT = None

NSPLIT = int(os.environ.get("KERNEL_NSPLIT", "4"))
WBUFS = int(os.environ.get("KERNEL_WBUFS", "4"))


def _build_nc(reps=1):
    nc = bacc.Bacc("TRN2", target_bir_lowering=False, debug=False, num_devices=E)

    xT = nc.dram_tensor("xT", (D, T), BF16, kind="ExternalInput").ap()
    wg = nc.dram_tensor("wg", (D, H), BF16, kind="ExternalInput").ap()
    wu = nc.dram_tensor("wu", (D, H), BF16, kind="ExternalInput").ap()
    wd = nc.dram_tensor("wd", (H, D), BF16, kind="ExternalInput").ap()
    ag = nc.dram_tensor("ag", (D, RP), BF16, kind="ExternalInput").ap()
    bg = nc.dram_tensor("bg", (RP, H), BF16, kind="ExternalInput").ap()
    au = nc.dram_tensor("au", (D, RP), BF16, kind="ExternalInput").ap()
    bu = nc.dram_tensor("bu", (RP, H), BF16, kind="ExternalInput").ap()
    ad = nc.dram_tensor("ad", (H, RP), BF16, kind="ExternalInput").ap()
    bd = nc.dram_tensor("bd", (RP, D), BF16, kind="ExternalInput").ap()
    out = nc.dram_tensor("out", (D, T), F32, kind="ExternalOutput").ap()

    aps = dict(
        xT_r=xT.rearrange("(o p) t -> p o t", p=P),
        wg_r=wg.rearrange("(o p) h -> p o h", p=P),
        wu_r=wu.rearrange("(o p) h -> p o h", p=P),
        wd_r=wd.rearrange("(o p) d -> p o d", p=P),
        ag_r=ag.rearrange("(o p) r -> p o r", p=P),
        au_r=au.rearrange("(o p) r -> p o r", p=P),
        ad_r=ad.rearrange("(o p) r -> p o r", p=P),
        out_r=out.rearrange("(o p) t -> p o t", p=P),
        bg=bg, bu=bu, bd=bd,
    )

    with tile.TileContext(nc) as tc:
        with (
            tc.tile_pool(name="persist", bufs=1) as pp,
            tc.tile_pool(name="stage", bufs=3) as sp,
            tc.tile_pool(name="wpool", bufs=WBUFS) as wp,
            tc.tile_pool(name="lslab", bufs=2) as lp,
            tc.tile_pool(name="psum", bufs=8, space="PSUM") as psp,
        ):
            for rep in range(reps):
                _emit(nc, tc, pp, sp, wp, lp, psp, aps, rep)

    nc.compile()
    return nc


def _dma_split(nc, dst, src, n):
    """Split a [P, O, F] slab load into n dma_starts over the O axis."""
    n = max(1, min(n, NSPLIT)) if NSPLIT > 0 else 1
    o = dst.shape[1]
    step = o // n
    for i in range(n):
        nc.sync.dma_start(dst[:, ts(i, step), :], src[:, ts(i, step), :])


def _emit(nc, tc, pp, sp, wp, lp, psp, aps, rep):
    xT_r, wg_r, wu_r, wd_r = aps["xT_r"], aps["wg_r"], aps["wu_r"], aps["wd_r"]
    ag_r, au_r, ad_r = aps["ag_r"], aps["au_r"], aps["ad_r"]
    bg, bu, bd, out_r = aps["bg"], aps["bu"], aps["bd"], aps["out_r"]

    hT_sb = pp.tile([P, HO, T], BF16, tag="hT")
    ag_sb = pp.tile([P, DO, RP], BF16, tag="ag")
    au_sb = pp.tile([P, DO, RP], BF16, tag="au")
    ad_sb = pp.tile([P, HO, RP], BF16, tag="ad")
    bd_sb = pp.tile([RP, D], BF16, tag="bd")
    aTg_sb = pp.tile([RP, T], BF16, tag="aTg")
    aTu_sb = pp.tile([RP, T], BF16, tag="aTu")
    aTd_sb = pp.tile([RP, T], BF16, tag="aTd")

    nc.sync.dma_start(ag_sb[:], ag_r[:])
    nc.sync.dma_start(au_sb[:], au_r[:])
    nc.sync.dma_start(ad_sb[:], ad_r[:])
    nc.sync.dma_start(bd_sb[:], bd[:])

    with tc.tile_pool(name=f"xpool{rep}", bufs=1) as xp:
        xT_sb = xp.tile([P, DO, T], BF16, tag="xT")
        _dma_split(nc, xT_sb, xT_r, 4)

        # aT = (x @ A)^T for gate/up (scale folded into B on host)
        for a_sb, aT_sb in ((ag_sb, aTg_sb), (au_sb, aTu_sb)):
            pa0 = psp.tile([RP, 512], F32, tag="mm")
            pa1 = psp.tile([RP, 512], F32, tag="mm")
            for o in range(DO):
                st, sp_ = (o == 0), (o == DO - 1)
                nc.tensor.matmul(pa0[:], a_sb[:, o, :], xT_sb[:, o, 0:512],
                                 start=st, stop=sp_)
                nc.tensor.matmul(pa1[:], a_sb[:, o, :], xT_sb[:, o, 512:1024],
                                 start=st, stop=sp_)
            nc.vector.tensor_copy(aT_sb[:, 0:512], pa0[:])
            nc.vector.tensor_copy(aT_sb[:, 512:1024], pa1[:])

        # layer 1: hT[h, t] = silu(gate) * up; lhsT paired over t-halves
        for j in range(H // 512):
            wg_t = wp.tile([P, DO, 512], BF16, tag="w")
            _dma_split(nc, wg_t, wg_r[:, :, ts(j, 512)], 4)
            wu_t = wp.tile([P, DO, 512], BF16, tag="w")
            _dma_split(nc, wu_t, wu_r[:, :, ts(j, 512)], 4)
            bg_t = lp.tile([RP, 512], BF16, tag="bgj")
            nc.sync.dma_start(bg_t[:], bg[:, ts(j, 512)])
            bu_t = lp.tile([RP, 512], BF16, tag="buj")
            nc.sync.dma_start(bu_t[:], bu[:, ts(j, 512)])
            for hsub in range(4):
                hc = j * 4 + hsub

                def l1_proj(w_t, b_t, aT_sb):
                    p0 = psp.tile([P, 512], F32, tag="mm")
                    p1 = psp.tile([P, 512], F32, tag="mm")
                    for o in range(DO):
                        st = (o == 0)
                        nc.tensor.matmul(p0[:], w_t[:, o, ts(hsub, P)],
                                         xT_sb[:, o, 0:512],
                                         start=st, stop=False)
                        nc.tensor.matmul(p1[:], w_t[:, o, ts(hsub, P)],
                                         xT_sb[:, o, 512:1024],
                                         start=st, stop=False)
                    nc.tensor.matmul(p0[:], b_t[:, ts(hsub, P)],
                                     aT_sb[:, 0:512], start=False, stop=True)
                    nc.tensor.matmul(p1[:], b_t[:, ts(hsub, P)],
                                     aT_sb[:, 512:1024], start=False, stop=True)
                    return p0, p1

                pg0, pg1 = l1_proj(wg_t, bg_t, aTg_sb)
                pu0, pu1 = l1_proj(wu_t, bu_t, aTu_sb)
                for t, pg_, pu_ in ((0, pg0, pu0), (1, pg1, pu1)):
                    g_act = sp.tile([P, 512], F32, tag="gact")
                    nc.scalar.activation(
                        g_act[:], pg_[:], mybir.ActivationFunctionType.Silu)
                    nc.vector.tensor_mul(
                        hT_sb[:, hc, ts(t, 512)], g_act[:], pu_[:])

    # aTd = (h @ Ad)^T, lhsT paired over t-halves
    pa0 = psp.tile([RP, 512], F32, tag="mm")
    pa1 = psp.tile([RP, 512], F32, tag="mm")
    for hc in range(HO):
        st, sp_ = (hc == 0), (hc == HO - 1)
        nc.tensor.matmul(pa0[:], ad_sb[:, hc, :], hT_sb[:, hc, 0:512],
                         start=st, stop=sp_)
        nc.tensor.matmul(pa1[:], ad_sb[:, hc, :], hT_sb[:, hc, 512:1024],
                         start=st, stop=sp_)
    nc.vector.tensor_copy(aTd_sb[:, 0:512], pa0[:])
    nc.vector.tensor_copy(aTd_sb[:, 512:1024], pa1[:])

    # layer 2: outT[d, t] = (h @ Wd + lora)^T; weight slices stationary,
    # paired over t-halves.
    for k in range(D // 512):
        s0 = wp.tile([P, DO, 512], BF16, tag="w")
        _dma_split(nc, s0, wd_r[:, 0:16, ts(k, 512)], 4)
        s1 = wp.tile([P, DO, 512], BF16, tag="w")
        _dma_split(nc, s1, wd_r[:, 16:32, ts(k, 512)], 4)
        for dsub in range(4):
            dd = k * 4 + dsub  # global 128-wide d-chunk
            po0 = psp.tile([P, 512], F32, tag="mm")
            po1 = psp.tile([P, 512], F32, tag="mm")
            for hc in range(HO):
                st = (hc == 0)
                lhsT = (s0 if hc < 16 else s1)[:, hc % 16, ts(dsub, P)]
                nc.tensor.matmul(po0[:], lhsT, hT_sb[:, hc, 0:512],
                                 start=st, stop=False)
                nc.tensor.matmul(po1[:], lhsT, hT_sb[:, hc, 512:1024],
                                 start=st, stop=False)
            nc.tensor.matmul(po0[:], bd_sb[:, ts(dd, P)], aTd_sb[:, 0:512],
                             start=False, stop=True)
            nc.tensor.matmul(po1[:], bd_sb[:, ts(dd, P)], aTd_sb[:, 512:1024],
                             start=False, stop=True)
            for t, po_ in ((0, po0), (1, po1)):
                o_t = sp.tile([P, 512], F32, tag="ostage")
                nc.scalar.copy(o_t[:], po_[:])
                nc.sync.dma_start(out_r[:, dd, ts(t, 512)], o_t[:])


def _build_nc2(reps=1, wsplit=2, xsplit=4, preload_b=False, wbufs=None,
               out_bf16=False):
    """v2: host-swizzled DRAM layouts -> every DMA descriptor is a fat
    contiguous per-partition line (8-32KB vs 1-2KB in v1).
    v3 = wsplit=1, xsplit=1, preload_b=True, wbufs=3 (descriptor-lean)."""
    nc = bacc.Bacc("TRN2", target_bir_lowering=False, debug=False, num_devices=E)

    HJ = H // 512  # 8 weight slabs for layer 1
    xTd = nc.dram_tensor("xTd", (P, DO * T), BF16, kind="ExternalInput").ap()
    wgd = nc.dram_tensor("wgd", (HJ, P, DO * 512), BF16, kind="ExternalInput").ap()
    wud = nc.dram_tensor("wud", (HJ, P, DO * 512), BF16, kind="ExternalInput").ap()
    wdd = nc.dram_tensor("wdd", (4, 2, P, 16 * 512), BF16, kind="ExternalInput").ap()
    agd = nc.dram_tensor("agd", (P, DO * RP), BF16, kind="ExternalInput").ap()
    aud = nc.dram_tensor("aud", (P, DO * RP), BF16, kind="ExternalInput").ap()
    add = nc.dram_tensor("add", (P, HO * RP), BF16, kind="ExternalInput").ap()
    bg = nc.dram_tensor("bg", (RP, H), BF16, kind="ExternalInput").ap()
    bu = nc.dram_tensor("bu", (RP, H), BF16, kind="ExternalInput").ap()
    bd = nc.dram_tensor("bd", (RP, D), BF16, kind="ExternalInput").ap()
    outd = nc.dram_tensor("outd", (4, 2, P, 2 * T),
                          BF16 if out_bf16 else F32,
                          kind="ExternalOutput").ap()

    aps = dict(
        xT_r=xTd.rearrange("p (o t) -> p o t", o=DO),
        wg_r=wgd.rearrange("j p (o c) -> j p o c", o=DO),
        wu_r=wud.rearrange("j p (o c) -> j p o c", o=DO),
        wd_r=wdd.rearrange("k h p (o c) -> k h p o c", o=16),
        ag_r=agd.rearrange("p (o r) -> p o r", o=DO),
        au_r=aud.rearrange("p (o r) -> p o r", o=DO),
        ad_r=add.rearrange("p (o r) -> p o r", o=HO),
        out_r=outd.rearrange("k h p (d t) -> k h p d t", d=2),
        bg=bg, bu=bu, bd=bd,
    )

    with tile.TileContext(nc) as tc:
        with (
            tc.tile_pool(name="persist", bufs=1) as pp,
            tc.tile_pool(name="stage", bufs=3) as sp,
            tc.tile_pool(name="wpool", bufs=wbufs or WBUFS) as wp,
            tc.tile_pool(name="lslab", bufs=2) as lp,
            tc.tile_pool(name="ostage", bufs=2) as op,
            tc.tile_pool(name="psum", bufs=8, space="PSUM") as psp,
        ):
            for rep in range(reps):
                _emit2(nc, tc, pp, sp, wp, lp, op, psp, aps, rep, wsplit,
                       xsplit, preload_b, out_bf16)

    nc.compile()
    return nc


def _dma_osplit(nc, dst, src, n):
    """Split a [P, O, F] slab load into n dma_starts over the O axis."""
    o = dst.shape[1]
    n = max(1, min(n, o))
    step = o // n
    for i in range(n):
        nc.sync.dma_start(dst[:, ts(i, step), :], src[:, ts(i, step), :])


def _emit2(nc, tc, pp, sp, wp, lp, op, psp, aps, rep, wsplit,
           xsplit=4, preload_b=False, out_bf16=False):
    xT_r, wg_r, wu_r, wd_r = aps["xT_r"], aps["wg_r"], aps["wu_r"], aps["wd_r"]
    ag_r, au_r, ad_r = aps["ag_r"], aps["au_r"], aps["ad_r"]
    bg, bu, bd, out_r = aps["bg"], aps["bu"], aps["bd"], aps["out_r"]

    hT_sb = pp.tile([P, HO, T], BF16, tag="hT")
    ag_sb = pp.tile([P, DO, RP], BF16, tag="ag")
    au_sb = pp.tile([P, DO, RP], BF16, tag="au")
    ad_sb = pp.tile([P, HO, RP], BF16, tag="ad")
    bd_sb = pp.tile([RP, D], BF16, tag="bd")
    aTg_sb = pp.tile([RP, T], BF16, tag="aTg")
    aTu_sb = pp.tile([RP, T], BF16, tag="aTu")
    aTd_sb = pp.tile([RP, T], BF16, tag="aTd")

    nc.sync.dma_start(ag_sb[:], ag_r[:])
    nc.sync.dma_start(au_sb[:], au_r[:])
    nc.sync.dma_start(ad_sb[:], ad_r[:])
    nc.sync.dma_start(bd_sb[:], bd[:])
    bg_sb = bu_sb = None
    if preload_b:
        bg_sb = pp.tile([RP, H], BF16, tag="bgall")
        bu_sb = pp.tile([RP, H], BF16, tag="buall")
        nc.sync.dma_start(bg_sb[:], bg[:])
        nc.sync.dma_start(bu_sb[:], bu[:])

    with tc.tile_pool(name=f"xpool{rep}", bufs=1) as xp:
        xT_sb = xp.tile([P, DO, T], BF16, tag="xT")
        _dma_osplit(nc, xT_sb, xT_r, xsplit)

        # aT = (x @ A)^T for gate/up (scale folded into B on host)
        for a_sb, aT_sb in ((ag_sb, aTg_sb), (au_sb, aTu_sb)):
            pa0 = psp.tile([RP, 512], F32, tag="mm")
            pa1 = psp.tile([RP, 512], F32, tag="mm")
            for o in range(DO):
                st, sp_ = (o == 0), (o == DO - 1)
                nc.tensor.matmul(pa0[:], a_sb[:, o, :], xT_sb[:, o, 0:512],
                                 start=st, stop=sp_)
                nc.tensor.matmul(pa1[:], a_sb[:, o, :], xT_sb[:, o, 512:1024],
                                 start=st, stop=sp_)
            nc.vector.tensor_copy(aT_sb[:, 0:512], pa0[:])
            nc.vector.tensor_copy(aT_sb[:, 512:1024], pa1[:])

        # layer 1: hT[h, t] = silu(gate) * up; lhsT paired over t-halves
        for j in range(H // 512):
            wg_t = wp.tile([P, DO, 512], BF16, tag="w")
            _dma_osplit(nc, wg_t, wg_r[j], wsplit)
            wu_t = wp.tile([P, DO, 512], BF16, tag="w")
            _dma_osplit(nc, wu_t, wu_r[j], wsplit)
            if preload_b:
                bg_t, bu_t, boff = bg_sb, bu_sb, j * 512
            else:
                bg_t = lp.tile([RP, 512], BF16, tag="bgj")
                nc.sync.dma_start(bg_t[:], bg[:, ts(j, 512)])
                bu_t = lp.tile([RP, 512], BF16, tag="buj")
                nc.sync.dma_start(bu_t[:], bu[:, ts(j, 512)])
                boff = 0
            for hsub in range(4):
                hc = j * 4 + hsub

                def l1_proj(w_t, b_t, aT_sb):
                    bs = boff + hsub * P
                    p0 = psp.tile([P, 512], F32, tag="mm")
                    p1 = psp.tile([P, 512], F32, tag="mm")
                    for o in range(DO):
                        st = (o == 0)
                        nc.tensor.matmul(p0[:], w_t[:, o, ts(hsub, P)],
                                         xT_sb[:, o, 0:512],
                                         start=st, stop=False)
                        nc.tensor.matmul(p1[:], w_t[:, o, ts(hsub, P)],
                                         xT_sb[:, o, 512:1024],
                                         start=st, stop=False)
                    nc.tensor.matmul(p0[:], b_t[:, bs:bs + P],
                                     aT_sb[:, 0:512], start=False, stop=True)
                    nc.tensor.matmul(p1[:], b_t[:, bs:bs + P],
                                     aT_sb[:, 512:1024], start=False, stop=True)
                    return p0, p1

                pg0, pg1 = l1_proj(wg_t, bg_t, aTg_sb)
                pu0, pu1 = l1_proj(wu_t, bu_t, aTu_sb)
                for t, pg_, pu_ in ((0, pg0, pu0), (1, pg1, pu1)):
                    g_act = sp.tile([P, 512], F32, tag="gact")
                    nc.scalar.activation(
                        g_act[:], pg_[:], mybir.ActivationFunctionType.Silu)
                    nc.vector.tensor_mul(
                        hT_sb[:, hc, ts(t, 512)], g_act[:], pu_[:])

    # aTd = (h @ Ad)^T, lhsT paired over t-halves
    pa0 = psp.tile([RP, 512], F32, tag="mm")
    pa1 = psp.tile([RP, 512], F32, tag="mm")
    for hc in range(HO):
        st, sp_ = (hc == 0), (hc == HO - 1)
        nc.tensor.matmul(pa0[:], ad_sb[:, hc, :], hT_sb[:, hc, 0:512],
                         start=st, stop=sp_)
        nc.tensor.matmul(pa1[:], ad_sb[:, hc, :], hT_sb[:, hc, 512:1024],
                         start=st, stop=sp_)
    nc.vector.tensor_copy(aTd_sb[:, 0:512], pa0[:])
    nc.vector.tensor_copy(aTd_sb[:, 512:1024], pa1[:])

    # layer 2: outT[d, t] = (h @ Wd + lora)^T; weight slices stationary,
    # paired over t-halves. Output staged as [P, 2, T] then one fat store.
    for k in range(D // 512):
        s0 = wp.tile([P, 16, 512], BF16, tag="w")
        _dma_osplit(nc, s0, wd_r[k, 0], wsplit)
        s1 = wp.tile([P, 16, 512], BF16, tag="w")
        _dma_osplit(nc, s1, wd_r[k, 1], wsplit)
        o_t = None
        for dsub in range(4):
            dd = k * 4 + dsub  # global 128-wide d-chunk
            if dsub % 2 == 0:
                o_t = op.tile([P, 2, T], BF16 if out_bf16 else F32,
                              tag="ostage")
            po0 = psp.tile([P, 512], F32, tag="mm")
            po1 = psp.tile([P, 512], F32, tag="mm")
            for hc in range(HO):
                st = (hc == 0)
                lhsT = (s0 if hc < 16 else s1)[:, hc % 16, ts(dsub, P)]
                nc.tensor.matmul(po0[:], lhsT, hT_sb[:, hc, 0:512],
                                 start=st, stop=False)
                nc.tensor.matmul(po1[:], lhsT, hT_sb[:, hc, 512:1024],
                                 start=st, stop=False)
            nc.tensor.matmul(po0[:], bd_sb[:, ts(dd, P)], aTd_sb[:, 0:512],
                             start=False, stop=True)
            nc.tensor.matmul(po1[:], bd_sb[:, ts(dd, P)], aTd_sb[:, 512:1024],
                             start=False, stop=True)
            nc.scalar.copy(o_t[:, dsub % 2, 0:512], po0[:])
            nc.scalar.copy(o_t[:, dsub % 2, 512:1024], po1[:])
            if dsub % 2 == 1:
                nc.sync.dma_start(out_r[k, dsub // 2], o_t[:])


def make_in_maps2(x, gate_proj, up_proj, down_proj, lga, lgb, lua, lub,
                  lda, ldb):
    """Host-side shard/cast/swizzle prep for the v2 fat-descriptor layout."""
    bf = ml_dtypes.bfloat16
    scale = ALPHA / R
    x = np.asarray(x, np.float32).reshape(E, T, D)

    def pad_b(b):
        o = np.zeros((RP, b.shape[1]), np.float32)
        o[:R] = scale * b
        return o.astype(bf)

    def sw_a(a):  # [in, R] -> [P, (o R)] padded to RP
        i = a.shape[0]
        o = np.zeros((i, RP), np.float32)
        o[:, :R] = a
        return np.ascontiguousarray(
            o.reshape(i // P, P, RP).transpose(1, 0, 2).reshape(P, -1)
        ).astype(bf)

    def sw_w1(w):  # [D, H] -> [HJ, P, (o c)]
        return np.ascontiguousarray(
            w.reshape(DO, P, H // 512, 512).transpose(2, 1, 0, 3)
            .reshape(H // 512, P, DO * 512)).astype(bf)

    def sw_wd(w):  # [H, D] -> [4, 2, P, (o c)]
        return np.ascontiguousarray(
            w.reshape(2, 16, P, 4, 512).transpose(3, 0, 2, 1, 4)
            .reshape(4, 2, P, 16 * 512)).astype(bf)

    in_maps = []
    for e in range(E):
        xT = np.ascontiguousarray(x[e].T)  # [D, T]
        in_maps.append({
            "xTd": np.ascontiguousarray(
                xT.reshape(DO, P, T).transpose(1, 0, 2).reshape(P, DO * T)
            ).astype(bf),
            "wgd": sw_w1(np.asarray(gate_proj[e], np.float32)),
            "wud": sw_w1(np.asarray(up_proj[e], np.float32)),
            "wdd": sw_wd(np.asarray(down_proj[e], np.float32)),
            "agd": sw_a(np.asarray(lga[e], np.float32)),
            "aud": sw_a(np.asarray(lua[e], np.float32)),
            "add": sw_a(np.asarray(lda[e], np.float32)),
            "bg": pad_b(np.asarray(lgb[e], np.float32)),
            "bu": pad_b(np.asarray(lub[e], np.float32)),
            "bd": pad_b(np.asarray(ldb[e], np.float32)),
        })
    return in_maps


def unswizzle_out2(outd):
    """outd [4, 2, P, 2*T] -> out [T, D] f32 for one expert."""
    o = outd.reshape(4, 2, P, 2, T).transpose(0, 1, 3, 2, 4).reshape(D, T)
    return np.ascontiguousarray(o.T).astype(np.float32)


def _build_tiny(big_unused=False, num_devices=E):
    """Trivial NEFF: one small DMA in, one out. Overhead probe."""
    nc = bacc.Bacc("TRN2", target_bir_lowering=False, debug=False,
                   num_devices=num_devices)
    if big_unused is True:
        HJ = H // 512
        xT = nc.dram_tensor("xTd", (P, DO * T), BF16, kind="ExternalInput").ap()
        nc.dram_tensor("wgd", (HJ, P, DO * 512), BF16, kind="ExternalInput")
        nc.dram_tensor("wud", (HJ, P, DO * 512), BF16, kind="ExternalInput")
        nc.dram_tensor("wdd", (4, 2, P, 16 * 512), BF16, kind="ExternalInput")
        out = nc.dram_tensor("outd", (4, 2, P, 2 * T), F32,
                             kind="ExternalOutput").ap()
        xr = xT.rearrange("p (o t) -> p o t", o=DO)
        our = out[0, 0]
        with tile.TileContext(nc) as tc:
            with tc.tile_pool(name="s", bufs=1) as spp:
                t0 = spp.tile([P, 128], BF16, tag="t")
                t1 = spp.tile([P, 128], F32, tag="t2")
                nc.sync.dma_start(t0[:], xr[:, 0, 0:128])
                nc.vector.tensor_copy(t1[:], t0[:])
                nc.sync.dma_start(our[:, 0:128], t1[:])
        nc.compile()
        return nc
    xT = nc.dram_tensor("xT", (D, T), BF16, kind="ExternalInput").ap()
    out = nc.dram_tensor("out", (D, T), F32, kind="ExternalOutput").ap()
    cw = None
    if big_unused == "const":
        cdata = np.zeros((24, P, 8192), ml_dtypes.bfloat16)  # 48MB const
        cw = nc.inline_tensor(cdata, name="cw").ap()
    xr = xT.rearrange("(o p) t -> p o t", p=P)
    our = out.rearrange("(o p) t -> p o t", p=P)
    with tile.TileContext(nc) as tc:
        with tc.tile_pool(name="s", bufs=1) as sp:
            t0 = sp.tile([P, 128], BF16, tag="t")
            t1 = sp.tile([P, 128], F32, tag="t2")
            nc.sync.dma_start(t0[:], xr[:, 0, 0:128])
            if cw is not None:
                tc0 = sp.tile([P, 128], BF16, tag="tc")
                tc1 = sp.tile([P, 128], F32, tag="tc2")
                nc.sync.dma_start(tc0[:], cw[0, :, 0:128])
                nc.vector.tensor_copy(tc1[:], tc0[:])
                nc.sync.dma_start(our[:, 1, 0:128], tc1[:])
            nc.vector.tensor_copy(t1[:], t0[:])
            nc.sync.dma_start(our[:, 0, 0:128], t1[:])
    nc.compile()
    return nc


def _build_nano(num_devices=E):
    """Minimal fully-read/fully-written NEFF: pure floor probe."""
    nc = bacc.Bacc("TRN2", target_bir_lowering=False, debug=False,
                   num_devices=num_devices)
    xi = nc.dram_tensor("xi", (P, 128), BF16, kind="ExternalInput").ap()
    out = nc.dram_tensor("out", (P, 128), F32, kind="ExternalOutput").ap()
    with tile.TileContext(nc) as tc:
        with tc.tile_pool(name="s", bufs=1) as spp:
            t0 = spp.tile([P, 128], BF16, tag="t")
            t1 = spp.tile([P, 128], F32, tag="t2")
            nc.sync.dma_start(t0[:], xi[:])
            nc.vector.tensor_copy(t1[:], t0[:])
            nc.sync.dma_start(out[:], t1[:])
    nc.compile()
    return nc


def build_variant(name):
    if name.startswith("nano"):
        n = int(name.split("@")[1]) if "@" in name else E
        return _build_nano(num_devices=n)
    if name == "tiny":
        return _build_tiny()
    if name == "tiny2":
        return _build_tiny(big_unused=True)
    if name == "tinyc":
        return _build_tiny(big_unused="const")
    if name.startswith("tiny@"):
        return _build_tiny(num_devices=int(name.split("@")[1]))
    if name == "v2":
        return _build_nc2()
    if name == "v3":
        return _build_nc2(wsplit=1, xsplit=1, preload_b=True, wbufs=3)
    if name == "v3b":
        return _build_nc2(wsplit=1, xsplit=1, preload_b=True, wbufs=3,
                          out_bf16=True)
    if name.startswith("v2r"):
        return _build_nc2(reps=int(name[3:]))
    raise KeyError(name)


def _get_nc():
    if not _NC_CACHE:
        _NC_CACHE.append(
            _build_nc2(wsplit=1, xsplit=1, preload_b=True, wbufs=3,
                       out_bf16=True))
    return _NC_CACHE[0]


def make_in_maps(x, gate_proj, up_proj, down_proj, lga, lgb, lua, lub, lda, ldb):
    """Host-side shard/cast prep, shared by kernel() and the bench harness."""
    bf = ml_dtypes.bfloat16
    scale = ALPHA / R
    x = np.asarray(x, np.float32).reshape(E, T, D)

    def pad_a(a):
        o = np.zeros((a.shape[0], RP), np.float32)
        o[:, :R] = a
        return o.astype(bf)

    def pad_b(b):
        o = np.zeros((RP, b.shape[1]), np.float32)
        o[:R] = scale * b
        return o.astype(bf)

    in_maps = []
    for e in range(E):
        in_maps.append({
            "xT": np.ascontiguousarray(x[e].T).astype(bf),
            "wg": np.asarray(gate_proj[e], np.float32).astype(bf),
            "wu": np.asarray(up_proj[e], np.float32).astype(bf),
            "wd": np.asarray(down_proj[e], np.float32).astype(bf),
            "ag": pad_a(np.asarray(lga[e], np.float32)),
            "bg": pad_b(np.asarray(lgb[e], np.float32)),
            "au": pad_a(np.asarray(lua[e], np.float32)),
            "bu": pad_b(np.asarray(lub[e], np.float32)),
            "ad": pad_a(np.asarray(lda[e], np.float32)),
            "bd": pad_b(np.asarray(ldb[e], np.float32)),
        })
    return in_maps


def kernel(x, num_tokens_per_expert, gate_proj, up_proj, down_proj,
           lora_gate_a, lora_gate_b, lora_up_a, lora_up_b,
           lora_down_a, lora_down_b):
    global LAST_RESULT
    in_maps = make_in_maps2(x, gate_proj, up_proj, down_proj,
                            lora_gate_a, lora_gate_b, lora_up_a, lora_up_b,
                            lora_down_a, lora_down_b)
    # The axon NTFF profile hook is unavailable in this container; force the
    # no-trace PJRT path regardless of ambient BASS_TRACE.
    os.environ["BASS_NEVER_TRACE"] = "1"
    nc = _get_nc()
    res = run_bass_kernel_spmd(nc, in_maps, core_ids=list(range(E)))
    LAST_RESULT = res
    # outputs are swizzled outT per expert; unswizzle back to [T, D]
    return np.concatenate(
        [unswizzle_out2(r["outd"]) for r in res.results], axis=0)

